# revision 11
# baseline (speedup 1.0000x reference)
"""DeepFM forward on Trainium2, 8 NeuronCores, data-parallel over batch.

Reference computes (B=512, n=512, K=4, H=128, n_pairs=130816):
    S  = fm_w @ fm_w.T
    fm = x[:, i1] * x[:, i2] * S[i1, i2]        # [B, n_pairs]
    h2 = relu(relu(x@w1+b1)@w2+b2)
    out = sigmoid(concat([fm, h2]) @ wo + bo)

The fm @ wo[:n_pairs] contraction is the bilinear form
    t1[b] = x[b]^T A x[b],   A[i,j] = S[i,j] * Wp[i,j]  (strictly upper)
where Wp is wo[:n_pairs] scattered into the upper triangle of a [n, n]
matrix (a pure re-layout of wo done on host; indices are static). Since
S = fm_w @ fm_w.T has rank 4, this further factors as
    t1[b] = sum_t z_t[b]^T Wp z_t[b],  z_t = x * fm_w[:, t]
so the device never materializes S or A: Wp is used directly as the
matmul operand and the rank-4 scaling is cheap per-partition DVE work.

Per-core program (batch shard = 64 columns, feature-on-partition layout,
bf16 operands / fp32 accumulation; t stacked along the free dim):
    Z_k[:, t, :]  = xT_k * fm_w[k-chunk, t]       (DVE tensor_scalar)
    VT_j = sum_k Wp[k128, j128]^T @ Z_k           (PE, k-major, N=256)
    Q_j  = VT_j * Z_j                             (DVE)
    T4   = sum_j ones^T @ Q_j                     (PE)  [1, 4, 64]
    h1   = max(w1^T @ xT + b1, 0)                 (PE+DVE)
    h2   = max(w2^T @ h1 + b2, 0)                 (PE+DVE)
    t2   = wo_h^T @ h2                            (PE)  [1, 64]
    out  = sigmoid(sum_t T4 + t2 + bo)            (DVE + ACT, table pre-warmed)
"""

import os
import sys

import numpy as np

for _p in ("/opt/trn_rl_repo", "/root/.axon_site/_ro/trn_rl_repo"):
    if os.path.isdir(_p) and _p not in sys.path:
        sys.path.insert(0, _p)

import ml_dtypes

import concourse.bass as bass
import concourse.tile as tile
from concourse import bacc, mybir
from concourse.bass import ts
from concourse.bass_utils import run_bass_kernel_spmd

F32 = mybir.dt.float32
BF16 = mybir.dt.bfloat16
AF = mybir.ActivationFunctionType
ALU = mybir.AluOpType

N = 512          # n_feat
KFM = 4          # fm embedding dim
H = 128          # mlp hidden
NP = N * (N - 1) // 2
B = 512
N_CORES = 8
BC = B // N_CORES  # 64 batch rows per core
NCH = N // 128     # 4 feature chunks

_IU1, _IU2 = np.triu_indices(N, k=1)

_program_cache = None


def _build_program():
    global _program_cache
    if _program_cache is not None:
        return _program_cache

    nc = bacc.Bacc(
        "TRN2", target_bir_lowering=False, debug=False, num_devices=N_CORES
    )
    xT_d = nc.declare_dram_parameter("xT", [N, BC], BF16, isOutput=False)
    wp_d = nc.declare_dram_parameter("wp", [N, N], BF16, isOutput=False)
    fmw_d = nc.declare_dram_parameter("fmw", [N, KFM], F32, isOutput=False)
    w1_d = nc.declare_dram_parameter("w1", [N, H], BF16, isOutput=False)
    w2_d = nc.declare_dram_parameter("w2", [H, H], BF16, isOutput=False)
    # pack columns: 0=b1, 1=b2, 2=wo_h, 3=[bo, 0...]
    pack_d = nc.declare_dram_parameter("pack", [H, 4], F32, isOutput=False)
    out_d = nc.declare_dram_parameter("out", [1, BC], F32, isOutput=True)

    with tile.TileContext(nc) as tc:
        with (
            tc.tile_pool(name="const", bufs=1) as cpool,
            tc.tile_pool(name="work", bufs=1) as wpool,
            tc.tile_pool(name="ps_v", bufs=1, space=bass.MemorySpace.PSUM) as vpool,
            tc.tile_pool(name="ps_h", bufs=1, space=bass.MemorySpace.PSUM) as hpool,
            tc.tile_pool(name="ps_t", bufs=1, space=bass.MemorySpace.PSUM) as tpool,
        ):
            # ---- loads: sync carries the critical path (xT, Wp) ----
            xt_sb = cpool.tile([128, NCH, BC], BF16)
            nc.sync.dma_start(
                xt_sb[:], xT_d[:, :].rearrange("(c p) b -> p c b", p=128)
            )
            wp_sb = cpool.tile([128, NCH, N], BF16)
            for h in range(2):  # two 256KB issues, halves land staggered
                nc.sync.dma_start(
                    wp_sb[:, ts(h, 2), :],
                    wp_d[:, :].rearrange("(c p) j -> p c j", p=128)[:, ts(h, 2), :],
                )
            fmw_sb = cpool.tile([128, NCH, KFM], F32)
            nc.scalar.dma_start(
                fmw_sb[:], fmw_d[:, :].rearrange("(c p) t -> p c t", p=128)
            )
            pack_sb = cpool.tile([H, 4], F32)
            nc.scalar.dma_start(pack_sb[:], pack_d[:, :])
            w1_sb = cpool.tile([128, NCH, H], BF16)
            nc.scalar.dma_start(
                w1_sb[:], w1_d[:, :].rearrange("(c p) h -> p c h", p=128)
            )
            w2_sb = cpool.tile([H, H], BF16)
            nc.scalar.dma_start(w2_sb[:], w2_d[:, :])

            # ---- constants + sigmoid ACT-table warm (off critical path) ----
            ones_sb = cpool.tile([128, 1], F32)
            nc.gpsimd.memset(ones_sb[:], 1.0)
            warm_in = cpool.tile([1, 1], F32)
            nc.gpsimd.memset(warm_in[:], 0.0)
            warm_out = cpool.tile([1, 1], F32)
            nc.scalar.activation(warm_out[:], warm_in[:], AF.Sigmoid, bias=0.0)

            # ---- Z_k[:, t, :] = xT_k scaled by fm_w column t (rank-4 trick) ----
            z_tiles = []
            for k in range(NCH):
                z_sb = wpool.tile([128, KFM, BC], BF16, name=f"z{k}", tag=f"z{k}")
                for t in range(KFM):
                    nc.vector.tensor_scalar_mul(
                        z_sb[:, t, :], xt_sb[:, k, :], fmw_sb[:, k, t : t + 1]
                    )
                z_tiles.append(z_sb)

            # ---- VT_j = sum_k Wp[k,j]^T @ Z_k (k-major: partials start on
            #      the first Wp half; t rides along the free dim, N=256) ----
            vt_tiles = [
                vpool.tile([128, KFM, BC], F32, name=f"vt{j}", tag=f"v{j}")
                for j in range(NCH)
            ]
            for k in range(NCH):
                for j in range(NCH):
                    nc.tensor.matmul(
                        vt_tiles[j][:], wp_sb[:, k, ts(j, 128)], z_tiles[k][:],
                        start=(k == 0), stop=(k == NCH - 1),
                    )

            # ---- MLP (scalar-engine-loaded weights; overlaps Wp path) ----
            h1_ps = hpool.tile([H, BC], F32)
            for k in range(NCH):
                nc.tensor.matmul(
                    h1_ps[:], w1_sb[:, k, :], xt_sb[:, k, :],
                    start=(k == 0), stop=(k == NCH - 1),
                )
            h1_sb = wpool.tile([H, BC], BF16)
            nc.vector.tensor_scalar(
                h1_sb[:], h1_ps[:], pack_sb[:, 0:1], 0.0,
                op0=ALU.add, op1=ALU.max,
            )
            h2_ps = hpool.tile([H, BC], F32)
            nc.tensor.matmul(h2_ps[:], w2_sb[:], h1_sb[:], start=True, stop=True)
            h2_sb = wpool.tile([H, BC], F32)
            nc.vector.tensor_scalar(
                h2_sb[:], h2_ps[:], pack_sb[:, 1:2], 0.0,
                op0=ALU.add, op1=ALU.max,
            )

            # ---- Q_j = VT_j * Z_j, then fold partitions via ones-matmul ----
            t4_ps = tpool.tile([1, KFM, BC], F32, tag="t4")
            q_tiles = []
            for j in range(NCH):
                q_sb = wpool.tile([128, KFM, BC], F32, name=f"q{j}", tag=f"q{j}")
                nc.vector.tensor_mul(q_sb[:], vt_tiles[j][:], z_tiles[j][:])
                q_tiles.append(q_sb)
                nc.tensor.matmul(
                    t4_ps[:], ones_sb[:], q_sb[:],
                    start=(j == 0), stop=(j == NCH - 1),
                )
            t2_ps = tpool.tile([1, BC], F32, tag="t2")
            nc.tensor.matmul(
                t2_ps[:], pack_sb[:, 2:3], h2_sb[:], start=True, stop=True
            )

            # ---- logit = sum_t T4 + t2, then sigmoid(. + bo) ----
            red = wpool.tile([1, BC], F32)
            nc.vector.tensor_reduce(
                red[:],
                t4_ps[:].rearrange("p t b -> p b t"),
                axis=mybir.AxisListType.X,
                op=ALU.add,
            )
            logit = wpool.tile([1, BC], F32)
            nc.vector.tensor_add(logit[:], red[:], t2_ps[:])
            out_sb = wpool.tile([1, BC], F32)
            nc.scalar.activation(
                out_sb[:], logit[:], AF.Sigmoid, bias=pack_sb[0:1, 3:4]
            )
            nc.sync.dma_start(out_d[:, :], out_sb[:])

    nc.compile()
    _program_cache = nc
    return nc


def _prep_inputs(x, fm_w, w1, b1, w2, b2, wo, bo):
    x = np.asarray(x, dtype=np.float32)
    fm_w = np.asarray(fm_w, dtype=np.float32)
    w1 = np.asarray(w1, dtype=np.float32)
    w2 = np.asarray(w2, dtype=np.float32)
    wo = np.asarray(wo, dtype=np.float32).reshape(NP + H)
    b1 = np.asarray(b1, dtype=np.float32).reshape(H)
    b2 = np.asarray(b2, dtype=np.float32).reshape(H)
    bo = np.asarray(bo, dtype=np.float32).reshape(1)

    # Scatter the pair weights into the strictly-upper triangle (static
    # index relayout; same (j1, j2>j1) row-major order as the reference).
    wp = np.zeros((N, N), dtype=np.float32)
    wp[_IU1, _IU2] = wo[:NP]

    pack = np.zeros((H, 4), dtype=np.float32)
    pack[:, 0] = b1
    pack[:, 1] = b2
    pack[:, 2] = wo[NP:]
    pack[0, 3] = bo[0]

    bf = ml_dtypes.bfloat16
    wp_bf = np.ascontiguousarray(wp.astype(bf))
    fmw_f32 = np.ascontiguousarray(fm_w)                # [512, 4] f32
    w1_bf = np.ascontiguousarray(w1.astype(bf))
    w2_bf = np.ascontiguousarray(w2.astype(bf))
    xT_bf = np.ascontiguousarray(x.T.astype(bf))        # [512, 512]

    shared = {
        "wp": wp_bf, "fmw": fmw_f32, "w1": w1_bf, "w2": w2_bf, "pack": pack,
    }
    in_maps = []
    for c in range(N_CORES):
        m = dict(shared)
        m["xT"] = np.ascontiguousarray(xT_bf[:, c * BC:(c + 1) * BC])
        in_maps.append(m)
    return in_maps


def run(inputs, **spmd_kwargs):
    """Build, run on 8 cores, return (output [512,1] f32, BassKernelResults)."""
    nc = _build_program()
    in_maps = _prep_inputs(**inputs)
    res = run_bass_kernel_spmd(nc, in_maps, list(range(N_CORES)), **spmd_kwargs)
    out = np.concatenate(
        [res.results[c]["out"].reshape(BC) for c in range(N_CORES)]
    ).reshape(B, 1).astype(np.float32)
    return out, res


def kernel(**inputs) -> np.ndarray:
    out, _ = run(inputs)
    return out


# revision 16
# speedup vs baseline: 1.0467x; 1.0467x over previous
"""DeepFM forward on Trainium2, 8 NeuronCores, data-parallel over batch.

Reference computes (B=512, n=512, K=4, H=128, n_pairs=130816):
    S  = fm_w @ fm_w.T
    fm = x[:, i1] * x[:, i2] * S[i1, i2]        # [B, n_pairs]
    h2 = relu(relu(x@w1+b1)@w2+b2)
    out = sigmoid(concat([fm, h2]) @ wo + bo)

The fm @ wo[:n_pairs] contraction is the bilinear form
    t1[b] = x[b]^T A x[b],   A[i,j] = S[i,j] * Wp[i,j]  (strictly upper)
where Wp is wo[:n_pairs] scattered into the upper triangle of a [n, n]
matrix (a pure re-layout of wo done on host; indices are static). Since
S = fm_w @ fm_w.T has rank 4, this further factors as
    t1[b] = sum_t z_t[b]^T Wp z_t[b],  z_t = x * fm_w[:, t]
so the device never materializes S or A: Wp is used directly as the
matmul operand and the rank-4 scaling is cheap per-partition DVE work.

Per-core program (batch shard = 64 columns, feature-on-partition layout,
bf16 operands / fp32 accumulation; t stacked along the free dim):
    Z_k[:, t, :]  = xT_k * fm_w[k-chunk, t]       (DVE tensor_scalar)
    VT_j = sum_k Wp[k128, j128]^T @ Z_k           (PE, k-major, N=256)
    Q_j  = VT_j * Z_j                             (DVE)
    T4   = sum_j ones^T @ Q_j                     (PE)  [1, 4, 64]
    h1   = max(w1^T @ xT + b1, 0)                 (PE+DVE)
    h2   = max(w2^T @ h1 + b2, 0)                 (PE+DVE)
    t2   = wo_h^T @ h2                            (PE)  [1, 64]
    out  = sigmoid(sum_t T4 + t2 + bo)            (DVE + ACT, table pre-warmed)
"""

import os
import sys

import numpy as np

for _p in ("/opt/trn_rl_repo", "/root/.axon_site/_ro/trn_rl_repo"):
    if os.path.isdir(_p) and _p not in sys.path:
        sys.path.insert(0, _p)

import ml_dtypes

import concourse.bass as bass
import concourse.tile as tile
from concourse import bacc, mybir
from concourse.bass import ts
from concourse.bass_utils import run_bass_kernel_spmd

F32 = mybir.dt.float32
BF16 = mybir.dt.bfloat16
AF = mybir.ActivationFunctionType
ALU = mybir.AluOpType

N = 512          # n_feat
KFM = 4          # fm embedding dim
H = 128          # mlp hidden
NP = N * (N - 1) // 2
B = 512
N_CORES = 8
BC = B // N_CORES  # 64 batch rows per core
NCH = N // 128     # 4 feature chunks
N_WARM = 16        # PE warm-up dummy matmuls

_IU1, _IU2 = np.triu_indices(N, k=1)

_program_cache = None


def _build_program():
    global _program_cache
    if _program_cache is not None:
        return _program_cache

    nc = bacc.Bacc(
        "TRN2", target_bir_lowering=False, debug=False, num_devices=N_CORES
    )
    xT_d = nc.declare_dram_parameter("xT", [N, BC], BF16, isOutput=False)
    wp_d = nc.declare_dram_parameter("wp", [N, N], BF16, isOutput=False)
    fmw_d = nc.declare_dram_parameter("fmw", [N, KFM], F32, isOutput=False)
    w1_d = nc.declare_dram_parameter("w1", [N, H], BF16, isOutput=False)
    w2_d = nc.declare_dram_parameter("w2", [H, H], BF16, isOutput=False)
    # pack columns: 0=b1, 1=b2, 2=wo_h, 3=[bo, 0...]
    pack_d = nc.declare_dram_parameter("pack", [H, 4], F32, isOutput=False)
    out_d = nc.declare_dram_parameter("out", [1, BC], F32, isOutput=True)

    with tile.TileContext(nc) as tc:
        with (
            tc.tile_pool(name="const", bufs=1) as cpool,
            tc.tile_pool(name="work", bufs=1) as wpool,
            tc.tile_pool(name="ps_v", bufs=1, space=bass.MemorySpace.PSUM) as vpool,
            tc.tile_pool(name="ps_h", bufs=1, space=bass.MemorySpace.PSUM) as hpool,
            tc.tile_pool(name="ps_t", bufs=1, space=bass.MemorySpace.PSUM) as tpool,
        ):
            # ---- loads: sync carries the critical path (Wp halves first) ----
            wp_sb = cpool.tile([128, NCH, N], BF16)
            for h in range(2):  # two 256KB issues, halves land staggered
                nc.sync.dma_start(
                    wp_sb[:, ts(h, 2), :],
                    wp_d[:, :].rearrange("(c p) j -> p c j", p=128)[:, ts(h, 2), :],
                )
            xt_sb = cpool.tile([128, NCH, BC], BF16)
            nc.scalar.dma_start(
                xt_sb[:], xT_d[:, :].rearrange("(c p) b -> p c b", p=128)
            )
            fmw_sb = cpool.tile([128, NCH, KFM], F32)
            nc.scalar.dma_start(
                fmw_sb[:], fmw_d[:, :].rearrange("(c p) t -> p c t", p=128)
            )
            w1_sb = cpool.tile([128, NCH, H], BF16)
            nc.scalar.dma_start(
                w1_sb[:], w1_d[:, :].rearrange("(c p) h -> p c h", p=128)
            )
            pack_sb = cpool.tile([H, 4], F32)
            nc.gpsimd.dma_start(pack_sb[:], pack_d[:, :])
            w2_sb = cpool.tile([H, H], BF16)
            nc.gpsimd.dma_start(w2_sb[:], w2_d[:, :])

            # ---- constants + sigmoid ACT-table warm (off critical path) ----
            ones_sb = cpool.tile([128, 1], BF16)
            nc.gpsimd.memset(ones_sb[:], 1.0)
            warm_in = cpool.tile([1, 1], F32)
            nc.gpsimd.memset(warm_in[:], 0.0)
            warm_out = cpool.tile([1, 1], F32)
            nc.scalar.activation(warm_out[:], warm_in[:], AF.Sigmoid, bias=0.0)

            # ---- PE HAM warm-up: keep TensorE busy through the DMA wait so
            #      the real matmuls run at the fast clock. Dummy matmuls on
            #      memset tiles, written into the vt banks as complete
            #      accumulation groups before their real use. ----
            dum_lhs = cpool.tile([128, 128], BF16)
            nc.gpsimd.memset(dum_lhs[:], 0.0)
            dum_rhs = cpool.tile([128, KFM * BC], BF16)
            nc.gpsimd.memset(dum_rhs[:], 0.0)

            # ---- Z_k[:, t, :] = xT_k scaled by fm_w column t (rank-4 trick) ----
            z_tiles = []
            for k in range(NCH):
                z_sb = wpool.tile([128, KFM, BC], BF16, name=f"z{k}", tag=f"z{k}")
                for t in range(KFM):
                    nc.vector.tensor_scalar_mul(
                        z_sb[:, t, :], xt_sb[:, k, :], fmw_sb[:, k, t : t + 1]
                    )
                z_tiles.append(z_sb)

            # ---- VT_j = sum_k Wp[k,j]^T @ Z_k (k-major: partials start on
            #      the first Wp half; t rides along the free dim, N=256) ----
            vt_tiles = [
                vpool.tile([128, KFM, BC], F32, name=f"vt{j}", tag=f"v{j}")
                for j in range(NCH)
            ]
            for d in range(N_WARM):
                nc.tensor.matmul(
                    vt_tiles[d % NCH][:], dum_lhs[:], dum_rhs[:],
                    start=True, stop=True,
                )
            for k in range(NCH):
                for j in range(NCH):
                    nc.tensor.matmul(
                        vt_tiles[j][:], wp_sb[:, k, ts(j, 128)], z_tiles[k][:],
                        start=(k == 0), stop=(k == NCH - 1),
                    )

            # ---- MLP (scalar-engine-loaded weights; overlaps Wp path) ----
            h1_ps = hpool.tile([H, BC], F32)
            for k in range(NCH):
                nc.tensor.matmul(
                    h1_ps[:], w1_sb[:, k, :], xt_sb[:, k, :],
                    start=(k == 0), stop=(k == NCH - 1),
                )
            h1_sb = wpool.tile([H, BC], BF16)
            nc.vector.tensor_scalar(
                h1_sb[:], h1_ps[:], pack_sb[:, 0:1], 0.0,
                op0=ALU.add, op1=ALU.max,
            )
            h2_ps = hpool.tile([H, BC], F32)
            nc.tensor.matmul(h2_ps[:], w2_sb[:], h1_sb[:], start=True, stop=True)
            h2_sb = wpool.tile([H, BC], F32)
            nc.vector.tensor_scalar(
                h2_sb[:], h2_ps[:], pack_sb[:, 1:2], 0.0,
                op0=ALU.add, op1=ALU.max,
            )

            # ---- Q_j = VT_j * Z_j, then fold partitions via ones-matmul ----
            t4_ps = tpool.tile([1, KFM, BC], F32, tag="t4")
            q_tiles = []
            for j in range(NCH):
                q_sb = wpool.tile([128, KFM, BC], BF16, name=f"q{j}", tag=f"q{j}")
                nc.vector.tensor_mul(q_sb[:], vt_tiles[j][:], z_tiles[j][:])
                q_tiles.append(q_sb)
                nc.tensor.matmul(
                    t4_ps[:], ones_sb[:], q_sb[:],
                    start=(j == 0), stop=(j == NCH - 1),
                )
            t2_ps = tpool.tile([1, BC], F32, tag="t2")
            nc.tensor.matmul(
                t2_ps[:], pack_sb[:, 2:3], h2_sb[:], start=True, stop=True
            )

            # ---- logit = sum_t T4 + t2, then sigmoid(. + bo) ----
            red = wpool.tile([1, BC], F32)
            nc.vector.tensor_reduce(
                red[:],
                t4_ps[:].rearrange("p t b -> p b t"),
                axis=mybir.AxisListType.X,
                op=ALU.add,
            )
            logit = wpool.tile([1, BC], F32)
            nc.vector.tensor_add(logit[:], red[:], t2_ps[:])
            out_sb = wpool.tile([1, BC], F32)
            nc.scalar.activation(
                out_sb[:], logit[:], AF.Sigmoid, bias=pack_sb[0:1, 3:4]
            )
            nc.sync.dma_start(out_d[:, :], out_sb[:])

    nc.compile()
    _program_cache = nc
    return nc


def _prep_inputs(x, fm_w, w1, b1, w2, b2, wo, bo):
    x = np.asarray(x, dtype=np.float32)
    fm_w = np.asarray(fm_w, dtype=np.float32)
    w1 = np.asarray(w1, dtype=np.float32)
    w2 = np.asarray(w2, dtype=np.float32)
    wo = np.asarray(wo, dtype=np.float32).reshape(NP + H)
    b1 = np.asarray(b1, dtype=np.float32).reshape(H)
    b2 = np.asarray(b2, dtype=np.float32).reshape(H)
    bo = np.asarray(bo, dtype=np.float32).reshape(1)

    # Scatter the pair weights into the strictly-upper triangle (static
    # index relayout; same (j1, j2>j1) row-major order as the reference).
    wp = np.zeros((N, N), dtype=np.float32)
    wp[_IU1, _IU2] = wo[:NP]

    pack = np.zeros((H, 4), dtype=np.float32)
    pack[:, 0] = b1
    pack[:, 1] = b2
    pack[:, 2] = wo[NP:]
    pack[0, 3] = bo[0]

    bf = ml_dtypes.bfloat16
    wp_bf = np.ascontiguousarray(wp.astype(bf))
    fmw_f32 = np.ascontiguousarray(fm_w)                # [512, 4] f32
    w1_bf = np.ascontiguousarray(w1.astype(bf))
    w2_bf = np.ascontiguousarray(w2.astype(bf))
    xT_bf = np.ascontiguousarray(x.T.astype(bf))        # [512, 512]

    shared = {
        "wp": wp_bf, "fmw": fmw_f32, "w1": w1_bf, "w2": w2_bf, "pack": pack,
    }
    in_maps = []
    for c in range(N_CORES):
        m = dict(shared)
        m["xT"] = np.ascontiguousarray(xT_bf[:, c * BC:(c + 1) * BC])
        in_maps.append(m)
    return in_maps


def run(inputs, **spmd_kwargs):
    """Build, run on 8 cores, return (output [512,1] f32, BassKernelResults)."""
    nc = _build_program()
    in_maps = _prep_inputs(**inputs)
    res = run_bass_kernel_spmd(nc, in_maps, list(range(N_CORES)), **spmd_kwargs)
    out = np.concatenate(
        [res.results[c]["out"].reshape(BC) for c in range(N_CORES)]
    ).reshape(B, 1).astype(np.float32)
    return out, res


def kernel(**inputs) -> np.ndarray:
    out, _ = run(inputs)
    return out


# revision 17
# speedup vs baseline: 1.0561x; 1.0089x over previous
"""DeepFM forward on Trainium2, 8 NeuronCores, data-parallel over batch.

Reference computes (B=512, n=512, K=4, H=128, n_pairs=130816):
    S  = fm_w @ fm_w.T
    fm = x[:, i1] * x[:, i2] * S[i1, i2]        # [B, n_pairs]
    h2 = relu(relu(x@w1+b1)@w2+b2)
    out = sigmoid(concat([fm, h2]) @ wo + bo)

The fm @ wo[:n_pairs] contraction is the bilinear form
    t1[b] = x[b]^T A x[b],   A[i,j] = S[i,j] * Wp[i,j]  (strictly upper)
where Wp is wo[:n_pairs] scattered into the upper triangle of a [n, n]
matrix (a pure re-layout of wo done on host; indices are static). Since
S = fm_w @ fm_w.T has rank 4, this further factors as
    t1[b] = sum_t z_t[b]^T Wp z_t[b],  z_t = x * fm_w[:, t]
so the device never materializes S or A: Wp is used directly as the
matmul operand and the rank-4 scaling is cheap per-partition DVE work.

All inputs are repacked on host into per-partition-contiguous [128, X]
SBUF images (feature chunk c of row r maps to partition r, column block
c) so each dma_start moves 128 large contiguous runs instead of 512 tiny
ones (the SDMA per-packet floor dominates latency otherwise).

Per-core program (batch shard = 64 columns, feature-on-partition layout,
bf16 operands / fp32 accumulation; t stacked along the free dim):
    Z_k[:, t, :]  = xT_k * fm_w[k-chunk, t]       (DVE tensor_scalar)
    VT_j = sum_k Wp[k128, j128]^T @ Z_k           (PE, k-major, N=256)
    Q_j  = VT_j * Z_j                             (DVE, bf16 out)
    T4   = sum_j ones^T @ Q_j                     (PE)  [1, 4, 64]
    h1   = max(w1^T @ xT + b1, 0)                 (PE+DVE)
    h2   = max(w2^T @ h1 + b2, 0)                 (PE+DVE)
    t2   = wo_h^T @ h2                            (PE)  [1, 64]
    out  = sigmoid(sum_t T4 + t2 + bo)            (DVE + ACT, table pre-warmed)

The PE is HAM-warmed with dummy matmuls on memset tiles during the DMA
wait so the real matmuls run at the fast clock.
"""

import os
import sys

import numpy as np

for _p in ("/opt/trn_rl_repo", "/root/.axon_site/_ro/trn_rl_repo"):
    if os.path.isdir(_p) and _p not in sys.path:
        sys.path.insert(0, _p)

import ml_dtypes

import concourse.bass as bass
import concourse.tile as tile
from concourse import bacc, mybir
from concourse.bass import ts
from concourse.bass_utils import run_bass_kernel_spmd

F32 = mybir.dt.float32
BF16 = mybir.dt.bfloat16
AF = mybir.ActivationFunctionType
ALU = mybir.AluOpType

N = 512          # n_feat
KFM = 4          # fm embedding dim
H = 128          # mlp hidden
NP = N * (N - 1) // 2
B = 512
N_CORES = 8
BC = B // N_CORES  # 64 batch rows per core
NCH = N // 128     # 4 feature chunks
N_WARM = 20        # PE warm-up dummy matmuls

# bf16 mega-pack column offsets: [xt | w1 | w2]
XT_OFF = 0                 # [128, 4*64]
W1_OFF = XT_OFF + NCH * BC   # [128, 4*128]
W2_OFF = W1_OFF + NCH * H    # [128, 128]
BF_COLS = W2_OFF + H       # 896
# f32 mega-pack column offsets: [fmw | b1 | b2 | woh | bo]
FM_OFF = 0                 # [128, 4*4]
PK_OFF = FM_OFF + NCH * KFM  # [128, 4]
F32_COLS = PK_OFF + 4      # 20

_IU1, _IU2 = np.triu_indices(N, k=1)

_program_cache = None


def _chunk_pack(a, cols):
    """[512, cols] row-major -> [128, 4*cols] with chunk c at column block c."""
    return np.ascontiguousarray(
        a.reshape(NCH, 128, cols).transpose(1, 0, 2).reshape(128, NCH * cols)
    )


def _build_program():
    global _program_cache
    if _program_cache is not None:
        return _program_cache

    nc = bacc.Bacc(
        "TRN2", target_bir_lowering=False, debug=False, num_devices=N_CORES
    )
    wp_d = nc.declare_dram_parameter("wp", [128, NCH * N], BF16, isOutput=False)
    bfm_d = nc.declare_dram_parameter("bfm", [128, BF_COLS], BF16, isOutput=False)
    f32_d = nc.declare_dram_parameter("f32m", [128, F32_COLS], F32, isOutput=False)
    out_d = nc.declare_dram_parameter("out", [1, BC], F32, isOutput=True)

    with tile.TileContext(nc) as tc:
        with (
            tc.tile_pool(name="const", bufs=1) as cpool,
            tc.tile_pool(name="work", bufs=1) as wpool,
            tc.tile_pool(name="ps_v", bufs=1, space=bass.MemorySpace.PSUM) as vpool,
            tc.tile_pool(name="ps_h", bufs=1, space=bass.MemorySpace.PSUM) as hpool,
            tc.tile_pool(name="ps_t", bufs=1, space=bass.MemorySpace.PSUM) as tpool,
        ):
            # ---- loads: 4 fat DMAs, per-partition contiguous ----
            wp_sb = cpool.tile([128, NCH * N], BF16)
            for hh in range(2):
                nc.sync.dma_start(
                    wp_sb[:, ts(hh, NCH * N // 2)], wp_d[:, ts(hh, NCH * N // 2)]
                )
            bfm_sb = cpool.tile([128, BF_COLS], BF16)
            nc.scalar.dma_start(bfm_sb[:], bfm_d[:, :])
            f32_sb = cpool.tile([128, F32_COLS], F32)
            nc.scalar.dma_start(f32_sb[:], f32_d[:, :])

            def xt(k):
                return bfm_sb[:, XT_OFF + k * BC : XT_OFF + (k + 1) * BC]

            def w1c(k):
                return bfm_sb[:, W1_OFF + k * H : W1_OFF + (k + 1) * H]

            w2_ap = bfm_sb[:, W2_OFF : W2_OFF + H]
            b1_ap = f32_sb[:, PK_OFF : PK_OFF + 1]
            b2_ap = f32_sb[:, PK_OFF + 1 : PK_OFF + 2]
            woh_ap = f32_sb[:, PK_OFF + 2 : PK_OFF + 3]
            bo_ap = f32_sb[0:1, PK_OFF + 3 : PK_OFF + 4]

            # ---- constants (Vector memsets — fast, idle early) ----
            dum_lhs = cpool.tile([128, 128], BF16)
            nc.vector.memset(dum_lhs[:], 0.0)
            dum_rhs = cpool.tile([128, KFM * BC], BF16)
            nc.vector.memset(dum_rhs[:], 0.0)
            ones_sb = cpool.tile([128, 1], BF16)
            nc.vector.memset(ones_sb[:], 1.0)
            warm_in = cpool.tile([1, 1], F32)
            nc.vector.memset(warm_in[:], 0.0)
            warm_out = cpool.tile([1, 1], F32)
            nc.scalar.activation(warm_out[:], warm_in[:], AF.Sigmoid, bias=0.0)

            # ---- PE HAM warm-up during the DMA wait ----
            vt_tiles = [
                vpool.tile([128, KFM, BC], F32, name=f"vt{j}", tag=f"v{j}")
                for j in range(NCH)
            ]
            for d in range(N_WARM):
                nc.tensor.matmul(
                    vt_tiles[d % NCH][:], dum_lhs[:], dum_rhs[:],
                    start=True, stop=True,
                )

            # ---- Z_k[:, t, :] = xT_k scaled by fm_w column t (rank-4) ----
            z_tiles = []
            for k in range(NCH):
                z_sb = wpool.tile([128, KFM, BC], BF16, name=f"z{k}", tag=f"z{k}")
                for t in range(KFM):
                    nc.vector.tensor_scalar_mul(
                        z_sb[:, t, :], xt(k),
                        f32_sb[:, FM_OFF + k * KFM + t : FM_OFF + k * KFM + t + 1],
                    )
                z_tiles.append(z_sb)

            # ---- VT_j = sum_k Wp[k,j]^T @ Z_k (k-major) ----
            for k in range(NCH):
                for j in range(NCH):
                    nc.tensor.matmul(
                        vt_tiles[j][:],
                        wp_sb[:, k * N + j * 128 : k * N + (j + 1) * 128],
                        z_tiles[k][:],
                        start=(k == 0), stop=(k == NCH - 1),
                    )

            # ---- MLP ----
            h1_ps = hpool.tile([H, BC], F32)
            for k in range(NCH):
                nc.tensor.matmul(
                    h1_ps[:], w1c(k), xt(k),
                    start=(k == 0), stop=(k == NCH - 1),
                )
            h1_sb = wpool.tile([H, BC], BF16)
            nc.vector.tensor_scalar(
                h1_sb[:], h1_ps[:], b1_ap, 0.0, op0=ALU.add, op1=ALU.max
            )
            h2_ps = hpool.tile([H, BC], F32)
            nc.tensor.matmul(h2_ps[:], w2_ap, h1_sb[:], start=True, stop=True)
            h2_sb = wpool.tile([H, BC], F32)
            nc.vector.tensor_scalar(
                h2_sb[:], h2_ps[:], b2_ap, 0.0, op0=ALU.add, op1=ALU.max
            )

            # ---- Q_j = VT_j * Z_j, fold partitions via ones-matmul ----
            t4_ps = tpool.tile([1, KFM, BC], F32, tag="t4")
            for j in range(NCH):
                q_sb = wpool.tile([128, KFM, BC], BF16, name=f"q{j}", tag=f"q{j}")
                nc.vector.tensor_mul(q_sb[:], vt_tiles[j][:], z_tiles[j][:])
                nc.tensor.matmul(
                    t4_ps[:], ones_sb[:], q_sb[:],
                    start=(j == 0), stop=(j == NCH - 1),
                )
            t2_ps = tpool.tile([1, BC], F32, tag="t2")
            nc.tensor.matmul(t2_ps[:], woh_ap, h2_sb[:], start=True, stop=True)

            # ---- logit = sum_t T4 + t2, then sigmoid(. + bo) ----
            red = wpool.tile([1, BC], F32)
            nc.vector.tensor_reduce(
                red[:],
                t4_ps[:].rearrange("p t b -> p b t"),
                axis=mybir.AxisListType.X,
                op=ALU.add,
            )
            logit = wpool.tile([1, BC], F32)
            nc.vector.tensor_add(logit[:], red[:], t2_ps[:])
            out_sb = wpool.tile([1, BC], F32)
            nc.scalar.activation(out_sb[:], logit[:], AF.Sigmoid, bias=bo_ap)
            nc.sync.dma_start(out_d[:, :], out_sb[:])

    nc.compile()
    _program_cache = nc
    return nc


def _prep_inputs(x, fm_w, w1, b1, w2, b2, wo, bo):
    x = np.asarray(x, dtype=np.float32)
    fm_w = np.asarray(fm_w, dtype=np.float32)
    w1 = np.asarray(w1, dtype=np.float32)
    w2 = np.asarray(w2, dtype=np.float32)
    wo = np.asarray(wo, dtype=np.float32).reshape(NP + H)
    b1 = np.asarray(b1, dtype=np.float32).reshape(H)
    b2 = np.asarray(b2, dtype=np.float32).reshape(H)
    bo = np.asarray(bo, dtype=np.float32).reshape(1)

    bf = ml_dtypes.bfloat16

    # Scatter the pair weights into the strictly-upper triangle (static
    # index relayout, same (j1, j2>j1) row-major order as the reference),
    # then chunk-pack to the SBUF image layout.
    wp = np.zeros((N, N), dtype=np.float32)
    wp[_IU1, _IU2] = wo[:NP]
    wp_img = _chunk_pack(wp.astype(bf), N)                      # [128, 2048]

    w1_img = _chunk_pack(w1.astype(bf), H)                      # [128, 512]
    fmw_img = _chunk_pack(fm_w, KFM)                            # [128, 16] f32

    f32_img = np.zeros((128, F32_COLS), dtype=np.float32)
    f32_img[:, FM_OFF : FM_OFF + NCH * KFM] = fmw_img
    f32_img[:, PK_OFF] = b1
    f32_img[:, PK_OFF + 1] = b2
    f32_img[:, PK_OFF + 2] = wo[NP:]
    f32_img[0, PK_OFF + 3] = bo[0]
    f32_img = np.ascontiguousarray(f32_img)

    xT = x.T.astype(bf)                                         # [512, 512]

    in_maps = []
    for c in range(N_CORES):
        xt_img = _chunk_pack(
            np.ascontiguousarray(xT[:, c * BC : (c + 1) * BC]), BC
        )                                                       # [128, 256]
        bfm = np.empty((128, BF_COLS), dtype=bf)
        bfm[:, XT_OFF : XT_OFF + NCH * BC] = xt_img
        bfm[:, W1_OFF : W1_OFF + NCH * H] = w1_img
        bfm[:, W2_OFF : W2_OFF + H] = w2.astype(bf)
        in_maps.append(
            {"wp": wp_img, "bfm": np.ascontiguousarray(bfm), "f32m": f32_img}
        )
    return in_maps


def run(inputs, **spmd_kwargs):
    """Build, run on 8 cores, return (output [512,1] f32, BassKernelResults)."""
    nc = _build_program()
    in_maps = _prep_inputs(**inputs)
    res = run_bass_kernel_spmd(nc, in_maps, list(range(N_CORES)), **spmd_kwargs)
    out = np.concatenate(
        [res.results[c]["out"].reshape(BC) for c in range(N_CORES)]
    ).reshape(B, 1).astype(np.float32)
    return out, res


def kernel(**inputs) -> np.ndarray:
    out, _ = run(inputs)
    return out


# revision 20
# speedup vs baseline: 1.0607x; 1.0044x over previous
"""DeepFM forward on Trainium2, 8 NeuronCores, data-parallel over batch.

Reference computes (B=512, n=512, K=4, H=128, n_pairs=130816):
    S  = fm_w @ fm_w.T
    fm = x[:, i1] * x[:, i2] * S[i1, i2]        # [B, n_pairs]
    h2 = relu(relu(x@w1+b1)@w2+b2)
    out = sigmoid(concat([fm, h2]) @ wo + bo)

The fm @ wo[:n_pairs] contraction is the bilinear form
    t1[b] = x[b]^T A x[b],   A[i,j] = S[i,j] * Wp[i,j]  (strictly upper)
where Wp is wo[:n_pairs] scattered into the upper triangle of a [n, n]
matrix (a pure re-layout of wo done on host; indices are static). Since
S = fm_w @ fm_w.T has rank 4, this further factors as
    t1[b] = sum_t z_t[b]^T Wp z_t[b],  z_t = x * fm_w[:, t]
so the device never materializes S or A: Wp is used directly as the
matmul operand and the rank-4 scaling is cheap per-partition DVE work.

All inputs are repacked on host into per-partition-contiguous [128, X]
SBUF images (feature chunk c of row r maps to partition r, column block
c) so each dma_start moves 128 large contiguous runs instead of 512 tiny
ones (the SDMA per-packet floor dominates latency otherwise).

Per-core program (batch shard = 64 columns, feature-on-partition layout,
bf16 operands / fp32 accumulation; t stacked along the free dim):
    Z_k[:, t, :]  = xT_k * fm_w[k-chunk, t]       (DVE tensor_scalar)
    VT_j = sum_k Wp[k128, j128]^T @ Z_k           (PE, k-major, N=256)
    Q_j  = VT_j * Z_j                             (DVE, bf16 out)
    T4   = sum_j ones^T @ Q_j                     (PE)  [1, 4, 64]
    h1   = max(w1^T @ xT + b1, 0)                 (PE+DVE)
    h2   = max(w2^T @ h1 + b2, 0)                 (PE+DVE)
    t2   = wo_h^T @ h2                            (PE)  [1, 64]
    out  = sigmoid(sum_t T4 + t2 + bo)            (DVE + ACT, table pre-warmed)

The PE is HAM-warmed with dummy matmuls on memset tiles during the DMA
wait so the real matmuls run at the fast clock.
"""

import os
import sys

import numpy as np

for _p in ("/opt/trn_rl_repo", "/root/.axon_site/_ro/trn_rl_repo"):
    if os.path.isdir(_p) and _p not in sys.path:
        sys.path.insert(0, _p)

import ml_dtypes

import concourse.bass as bass
import concourse.tile as tile
from concourse import bacc, mybir
from concourse.bass import ts
from concourse.bass_utils import run_bass_kernel_spmd

F32 = mybir.dt.float32
BF16 = mybir.dt.bfloat16
AF = mybir.ActivationFunctionType
ALU = mybir.AluOpType

N = 512          # n_feat
KFM = 4          # fm embedding dim
H = 128          # mlp hidden
NP = N * (N - 1) // 2
B = 512
N_CORES = 8
BC = B // N_CORES  # 64 batch rows per core
NCH = N // 128     # 4 feature chunks
N_WARM = 26        # PE warm-up dummy matmuls

# bf16 mega-pack column offsets: [xt | w1 | w2]
XT_OFF = 0                 # [128, 4*64]
W1_OFF = XT_OFF + NCH * BC   # [128, 4*128]
W2_OFF = W1_OFF + NCH * H    # [128, 128]
BF_COLS = W2_OFF + H       # 896
# f32 mega-pack column offsets: [fmw | b1 | b2 | woh | bo]
FM_OFF = 0                 # [128, 4*4]
PK_OFF = FM_OFF + NCH * KFM  # [128, 4]
F32_COLS = PK_OFF + 4      # 20

_IU1, _IU2 = np.triu_indices(N, k=1)

_program_cache = None


def _chunk_pack(a, cols):
    """[512, cols] row-major -> [128, 4*cols] with chunk c at column block c."""
    return np.ascontiguousarray(
        a.reshape(NCH, 128, cols).transpose(1, 0, 2).reshape(128, NCH * cols)
    )


def _build_program():
    global _program_cache
    if _program_cache is not None:
        return _program_cache

    nc = bacc.Bacc(
        "TRN2", target_bir_lowering=False, debug=False, num_devices=N_CORES
    )
    wp_d = nc.declare_dram_parameter("wp", [128, NCH * N], BF16, isOutput=False)
    bfm_d = nc.declare_dram_parameter("bfm", [128, BF_COLS], BF16, isOutput=False)
    f32_d = nc.declare_dram_parameter("f32m", [128, F32_COLS], F32, isOutput=False)
    out_d = nc.declare_dram_parameter("out", [1, BC], F32, isOutput=True)

    with tile.TileContext(nc) as tc:
        with (
            tc.tile_pool(name="const", bufs=1) as cpool,
            tc.tile_pool(name="work", bufs=1) as wpool,
            tc.tile_pool(name="ps_v", bufs=1, space=bass.MemorySpace.PSUM) as vpool,
            tc.tile_pool(name="ps_h", bufs=1, space=bass.MemorySpace.PSUM) as hpool,
            tc.tile_pool(name="ps_t", bufs=1, space=bass.MemorySpace.PSUM) as tpool,
        ):
            # ---- loads: 4 fat DMAs, per-partition contiguous ----
            wp_sb = cpool.tile([128, NCH * N], BF16)
            for hh in range(2):
                nc.sync.dma_start(
                    wp_sb[:, ts(hh, NCH * N // 2)], wp_d[:, ts(hh, NCH * N // 2)]
                )
            f32_sb = cpool.tile([128, F32_COLS], F32)
            nc.scalar.dma_start(f32_sb[:], f32_d[:, :])
            bfm_sb = cpool.tile([128, BF_COLS], BF16)
            nc.scalar.dma_start(bfm_sb[:], bfm_d[:, :])

            def xt(k):
                return bfm_sb[:, XT_OFF + k * BC : XT_OFF + (k + 1) * BC]

            def w1c(k):
                return bfm_sb[:, W1_OFF + k * H : W1_OFF + (k + 1) * H]

            w2_ap = bfm_sb[:, W2_OFF : W2_OFF + H]
            b1_ap = f32_sb[:, PK_OFF : PK_OFF + 1]
            b2_ap = f32_sb[:, PK_OFF + 1 : PK_OFF + 2]
            woh_ap = f32_sb[:, PK_OFF + 2 : PK_OFF + 3]
            bo_ap = f32_sb[0:1, PK_OFF + 3 : PK_OFF + 4]

            # ---- constants (Vector memsets — fast, idle early) ----
            dum_lhs = cpool.tile([128, 128], BF16)
            nc.vector.memset(dum_lhs[:], 0.0)
            dum_rhs = cpool.tile([128, KFM * BC], BF16)
            nc.vector.memset(dum_rhs[:], 0.0)
            ones_sb = cpool.tile([128, 1], BF16)
            nc.vector.memset(ones_sb[:], 1.0)
            warm_in = cpool.tile([1, 1], F32)
            nc.vector.memset(warm_in[:], 0.0)
            warm_out = cpool.tile([1, 1], F32)
            nc.scalar.activation(warm_out[:], warm_in[:], AF.Sigmoid, bias=0.0)

            # ---- PE HAM warm-up during the DMA wait ----
            vt_tiles = [
                vpool.tile([128, KFM, BC], F32, name=f"vt{j}", tag=f"v{j}")
                for j in range(NCH)
            ]
            for d in range(N_WARM):
                nc.tensor.matmul(
                    vt_tiles[d % NCH][:], dum_lhs[:], dum_rhs[:],
                    start=True, stop=True,
                )

            # ---- Z_k[:, t, :] = xT_k scaled by fm_w column t (rank-4).
            #      One broadcast tensor_tensor per chunk: xt repeats over t
            #      (stride-0), the fm_w column vector repeats over b. ----
            z_tiles = []
            for k in range(NCH):
                z_sb = wpool.tile([128, KFM, BC], BF16, name=f"z{k}", tag=f"z{k}")
                nc.vector.tensor_mul(
                    z_sb[:],
                    xt(k)[:, None, :].broadcast_to([128, KFM, BC]),
                    f32_sb[:, FM_OFF + k * KFM : FM_OFF + (k + 1) * KFM][
                        :, :, None
                    ].broadcast_to([128, KFM, BC]),
                )
                z_tiles.append(z_sb)

            # ---- VT_j = sum_k Wp[k,j]^T @ Z_k (k-major) ----
            for k in range(NCH):
                for j in range(NCH):
                    nc.tensor.matmul(
                        vt_tiles[j][:],
                        wp_sb[:, k * N + j * 128 : k * N + (j + 1) * 128],
                        z_tiles[k][:],
                        start=(k == 0), stop=(k == NCH - 1),
                    )

            # ---- MLP ----
            h1_ps = hpool.tile([H, BC], F32)
            for k in range(NCH):
                nc.tensor.matmul(
                    h1_ps[:], w1c(k), xt(k),
                    start=(k == 0), stop=(k == NCH - 1),
                )
            h1_sb = wpool.tile([H, BC], BF16)
            nc.vector.tensor_scalar(
                h1_sb[:], h1_ps[:], b1_ap, 0.0, op0=ALU.add, op1=ALU.max
            )
            h2_ps = hpool.tile([H, BC], F32)
            nc.tensor.matmul(h2_ps[:], w2_ap, h1_sb[:], start=True, stop=True)
            h2_sb = wpool.tile([H, BC], F32)
            nc.vector.tensor_scalar(
                h2_sb[:], h2_ps[:], b2_ap, 0.0, op0=ALU.add, op1=ALU.max
            )

            # ---- Q_j = VT_j * Z_j, fold partitions via ones-matmul ----
            t4_ps = tpool.tile([1, KFM, BC], F32, tag="t4")
            for j in range(NCH):
                q_sb = wpool.tile([128, KFM, BC], BF16, name=f"q{j}", tag=f"q{j}")
                nc.vector.tensor_mul(q_sb[:], vt_tiles[j][:], z_tiles[j][:])
                nc.tensor.matmul(
                    t4_ps[:], ones_sb[:], q_sb[:],
                    start=(j == 0), stop=(j == NCH - 1),
                )
            t2_ps = tpool.tile([1, BC], F32, tag="t2")
            nc.tensor.matmul(t2_ps[:], woh_ap, h2_sb[:], start=True, stop=True)

            # ---- logit = sum_t T4 + t2, then sigmoid(. + bo) ----
            red = wpool.tile([1, BC], F32)
            nc.vector.tensor_reduce(
                red[:],
                t4_ps[:].rearrange("p t b -> p b t"),
                axis=mybir.AxisListType.X,
                op=ALU.add,
            )
            logit = wpool.tile([1, BC], F32)
            nc.vector.tensor_add(logit[:], red[:], t2_ps[:])
            out_sb = wpool.tile([1, BC], F32)
            nc.scalar.activation(out_sb[:], logit[:], AF.Sigmoid, bias=bo_ap)
            nc.sync.dma_start(out_d[:, :], out_sb[:])

    nc.compile()
    _program_cache = nc
    return nc


def _prep_inputs(x, fm_w, w1, b1, w2, b2, wo, bo):
    x = np.asarray(x, dtype=np.float32)
    fm_w = np.asarray(fm_w, dtype=np.float32)
    w1 = np.asarray(w1, dtype=np.float32)
    w2 = np.asarray(w2, dtype=np.float32)
    wo = np.asarray(wo, dtype=np.float32).reshape(NP + H)
    b1 = np.asarray(b1, dtype=np.float32).reshape(H)
    b2 = np.asarray(b2, dtype=np.float32).reshape(H)
    bo = np.asarray(bo, dtype=np.float32).reshape(1)

    bf = ml_dtypes.bfloat16

    # Scatter the pair weights into the strictly-upper triangle (static
    # index relayout, same (j1, j2>j1) row-major order as the reference),
    # then chunk-pack to the SBUF image layout.
    wp = np.zeros((N, N), dtype=np.float32)
    wp[_IU1, _IU2] = wo[:NP]
    wp_img = _chunk_pack(wp.astype(bf), N)                      # [128, 2048]

    w1_img = _chunk_pack(w1.astype(bf), H)                      # [128, 512]
    fmw_img = _chunk_pack(fm_w, KFM)                            # [128, 16] f32

    f32_img = np.zeros((128, F32_COLS), dtype=np.float32)
    f32_img[:, FM_OFF : FM_OFF + NCH * KFM] = fmw_img
    f32_img[:, PK_OFF] = b1
    f32_img[:, PK_OFF + 1] = b2
    f32_img[:, PK_OFF + 2] = wo[NP:]
    f32_img[0, PK_OFF + 3] = bo[0]
    f32_img = np.ascontiguousarray(f32_img)

    xT = x.T.astype(bf)                                         # [512, 512]

    in_maps = []
    for c in range(N_CORES):
        xt_img = _chunk_pack(
            np.ascontiguousarray(xT[:, c * BC : (c + 1) * BC]), BC
        )                                                       # [128, 256]
        bfm = np.empty((128, BF_COLS), dtype=bf)
        bfm[:, XT_OFF : XT_OFF + NCH * BC] = xt_img
        bfm[:, W1_OFF : W1_OFF + NCH * H] = w1_img
        bfm[:, W2_OFF : W2_OFF + H] = w2.astype(bf)
        in_maps.append(
            {"wp": wp_img, "bfm": np.ascontiguousarray(bfm), "f32m": f32_img}
        )
    return in_maps


def run(inputs, **spmd_kwargs):
    """Build, run on 8 cores, return (output [512,1] f32, BassKernelResults)."""
    nc = _build_program()
    in_maps = _prep_inputs(**inputs)
    res = run_bass_kernel_spmd(nc, in_maps, list(range(N_CORES)), **spmd_kwargs)
    out = np.concatenate(
        [res.results[c]["out"].reshape(BC) for c in range(N_CORES)]
    ).reshape(B, 1).astype(np.float32)
    return out, res


def kernel(**inputs) -> np.ndarray:
    out, _ = run(inputs)
    return out


# revision 21
# speedup vs baseline: 1.1931x; 1.1249x over previous
"""DeepFM forward on Trainium2, 8 NeuronCores, data-parallel over batch.

Reference computes (B=512, n=512, K=4, H=128, n_pairs=130816):
    S  = fm_w @ fm_w.T
    fm = x[:, i1] * x[:, i2] * S[i1, i2]        # [B, n_pairs]
    h2 = relu(relu(x@w1+b1)@w2+b2)
    out = sigmoid(concat([fm, h2]) @ wo + bo)

The fm @ wo[:n_pairs] contraction is the bilinear form
    t1[b] = x[b]^T Wp' x[b]  with  Wp'[i,j] = S[i,j] * Wp[i,j]
where Wp is wo[:n_pairs] scattered into the strictly-upper triangle of a
[n, n] matrix (a pure re-layout of wo done on host; indices are static).
Since S = fm_w @ fm_w.T has rank 4, this factors as
    t1[b] = sum_t z_t[b]^T Wp z_t[b],  z_t = x * fm_w[:, t]
so the device never materializes S: Wp is used directly as the matmul
operand and the rank-4 scaling is cheap broadcast DVE work. Wp is
strictly upper triangular, so only the 10 upper-triangular 128x128
blocks are shipped and multiplied (the 6 lower blocks are zero).

All inputs are repacked on host into per-partition-contiguous [128, X]
SBUF images so each dma_start moves 128 fat contiguous runs (the SDMA
per-packet cost dominates latency otherwise).

Per-core program (batch shard = 64 columns, feature-on-partition layout,
bf16 operands / fp32 accumulation; t stacked along the free dim):
    Z_k[:, t, :]  = xT_k * fm_w[k-chunk, t]       (DVE broadcast mul)
    VT_j = sum_{k<=j} Wp[k128, j128]^T @ Z_k      (PE, j-major blocks)
    Q_j  = VT_j * Z_j                             (DVE, bf16 out)
    t    = sum_{j,t} ones^T @ Q_j[:,t,:] + wo_h^T @ h2   (PE psum accum) [1,64]
    h1   = max(w1^T @ xT + b1, 0)                 (PE+DVE)
    h2   = max(w2^T @ h1 + b2, 0)                 (PE+DVE)
    out  = sigmoid(t + bo)                        (ACT, table pre-warmed)

The PE is HAM-warmed with dummy matmuls on memset tiles during the DMA
wait so the real matmuls run closer to the fast clock.
"""

import os
import sys

import numpy as np

for _p in ("/opt/trn_rl_repo", "/root/.axon_site/_ro/trn_rl_repo"):
    if os.path.isdir(_p) and _p not in sys.path:
        sys.path.insert(0, _p)

import ml_dtypes

import concourse.bass as bass
import concourse.tile as tile
from concourse import bacc, mybir
from concourse.bass import ts
from concourse.bass_utils import run_bass_kernel_spmd

F32 = mybir.dt.float32
BF16 = mybir.dt.bfloat16
AF = mybir.ActivationFunctionType
ALU = mybir.AluOpType

N = 512          # n_feat
KFM = 4          # fm embedding dim
H = 128          # mlp hidden
NP = N * (N - 1) // 2
B = 512
N_CORES = 8
BC = B // N_CORES  # 64 batch rows per core
NCH = N // 128     # 4 feature chunks
N_WARM = 16        # PE warm-up dummy matmuls

# Upper-triangular 128x128 blocks of Wp in j-major order.
UBLOCKS = [(k, j) for j in range(NCH) for k in range(j + 1)]
UB_OFF = {kj: i * 128 for i, kj in enumerate(UBLOCKS)}  # column offset in image
WP_COLS = len(UBLOCKS) * 128  # 1280
WP_SPLIT = UB_OFF[(0, 3)]     # j0..j2 blocks first, then j3's

# f32 pack column offsets: [fmw | b1 | b2 | woh | bo]
FM_OFF = 0                 # [128, 4*4]
PK_OFF = FM_OFF + NCH * KFM  # [128, 4]
F32_COLS = PK_OFF + 4      # 20

_IU1, _IU2 = np.triu_indices(N, k=1)

_program_cache = None


def _chunk_pack(a, cols):
    """[512, cols] row-major -> [128, 4*cols] with chunk c at column block c."""
    return np.ascontiguousarray(
        a.reshape(NCH, 128, cols).transpose(1, 0, 2).reshape(128, NCH * cols)
    )


def _build_program():
    global _program_cache
    if _program_cache is not None:
        return _program_cache

    nc = bacc.Bacc(
        "TRN2", target_bir_lowering=False, debug=False, num_devices=N_CORES
    )
    wp_d = nc.declare_dram_parameter("wp", [128, WP_COLS], BF16, isOutput=False)
    xt_d = nc.declare_dram_parameter("xtm", [128, NCH * BC], BF16, isOutput=False)
    w12_d = nc.declare_dram_parameter(
        "w12", [128, NCH * H + H], BF16, isOutput=False
    )
    f32_d = nc.declare_dram_parameter("f32m", [128, F32_COLS], F32, isOutput=False)
    out_d = nc.declare_dram_parameter("out", [1, BC], F32, isOutput=True)

    with tile.TileContext(nc) as tc:
        with (
            tc.tile_pool(name="const", bufs=1) as cpool,
            tc.tile_pool(name="work", bufs=1) as wpool,
            tc.tile_pool(name="ps_v", bufs=1, space=bass.MemorySpace.PSUM) as vpool,
            tc.tile_pool(name="ps_h", bufs=1, space=bass.MemorySpace.PSUM) as hpool,
            tc.tile_pool(name="ps_t", bufs=1, space=bass.MemorySpace.PSUM) as tpool,
        ):
            # ---- loads. sync: Wp blocks then mlp weights; scalar: scalars+x ----
            wp_sb = cpool.tile([128, WP_COLS], BF16)
            nc.sync.dma_start(wp_sb[:, :WP_SPLIT], wp_d[:, :WP_SPLIT])
            nc.sync.dma_start(wp_sb[:, WP_SPLIT:], wp_d[:, WP_SPLIT:])
            w12_sb = cpool.tile([128, NCH * H + H], BF16)
            nc.sync.dma_start(w12_sb[:], w12_d[:, :])
            f32_sb = cpool.tile([128, F32_COLS], F32)
            nc.scalar.dma_start(f32_sb[:], f32_d[:, :])
            xt_sb = cpool.tile([128, NCH * BC], BF16)
            nc.scalar.dma_start(xt_sb[:], xt_d[:, :])

            def xt(k):
                return xt_sb[:, k * BC : (k + 1) * BC]

            def w1c(k):
                return w12_sb[:, k * H : (k + 1) * H]

            w2_ap = w12_sb[:, NCH * H : NCH * H + H]
            b1_ap = f32_sb[:, PK_OFF : PK_OFF + 1]
            b2_ap = f32_sb[:, PK_OFF + 1 : PK_OFF + 2]
            woh_ap = f32_sb[:, PK_OFF + 2 : PK_OFF + 3]
            bo_ap = f32_sb[0:1, PK_OFF + 3 : PK_OFF + 4]

            # ---- constants (Vector memsets — fast, idle early) ----
            dum_lhs = cpool.tile([128, 128], BF16)
            nc.vector.memset(dum_lhs[:], 0.0)
            dum_rhs = cpool.tile([128, KFM * BC], BF16)
            nc.vector.memset(dum_rhs[:], 0.0)
            ones_sb = cpool.tile([128, 1], BF16)
            nc.vector.memset(ones_sb[:], 1.0)
            warm_in = cpool.tile([1, 1], F32)
            nc.vector.memset(warm_in[:], 0.0)
            warm_out = cpool.tile([1, 1], F32)
            nc.scalar.activation(warm_out[:], warm_in[:], AF.Sigmoid, bias=0.0)

            # ---- PE HAM warm-up during the DMA wait ----
            vt_tiles = [
                vpool.tile([128, KFM, BC], F32, name=f"vt{j}", tag=f"v{j}")
                for j in range(NCH)
            ]
            for d in range(N_WARM):
                nc.tensor.matmul(
                    vt_tiles[d % NCH][:], dum_lhs[:], dum_rhs[:],
                    start=True, stop=True,
                )

            # ---- Z_k[:, t, :] = xT_k scaled by fm_w column t (rank-4).
            #      One broadcast tensor_tensor per chunk. ----
            z_tiles = []
            for k in range(NCH):
                z_sb = wpool.tile([128, KFM, BC], BF16, name=f"z{k}", tag=f"z{k}")
                nc.vector.tensor_mul(
                    z_sb[:],
                    xt(k)[:, None, :].broadcast_to([128, KFM, BC]),
                    f32_sb[:, FM_OFF + k * KFM : FM_OFF + (k + 1) * KFM][
                        :, :, None
                    ].broadcast_to([128, KFM, BC]),
                )
                z_tiles.append(z_sb)

            # ---- VT_j = sum_{k<=j} Wp[k,j]^T @ Z_k (upper blocks only) ----
            for j in range(NCH):
                for k in range(j + 1):
                    off = UB_OFF[(k, j)]
                    nc.tensor.matmul(
                        vt_tiles[j][:], wp_sb[:, off : off + 128], z_tiles[k][:],
                        start=(k == 0), stop=(k == j),
                    )

            # ---- MLP ----
            h1_ps = hpool.tile([H, BC], F32)
            for k in range(NCH):
                nc.tensor.matmul(
                    h1_ps[:], w1c(k), xt(k),
                    start=(k == 0), stop=(k == NCH - 1),
                )
            h1_sb = wpool.tile([H, BC], BF16)
            nc.vector.tensor_scalar(
                h1_sb[:], h1_ps[:], b1_ap, 0.0, op0=ALU.add, op1=ALU.max
            )
            h2_ps = hpool.tile([H, BC], F32)
            nc.tensor.matmul(h2_ps[:], w2_ap, h1_sb[:], start=True, stop=True)
            h2_sb = wpool.tile([H, BC], F32)
            nc.vector.tensor_scalar(
                h2_sb[:], h2_ps[:], b2_ap, 0.0, op0=ALU.add, op1=ALU.max
            )

            # ---- Q_j = VT_j * Z_j; fold partitions AND t via one psum
            #      accumulation group of [1, 64] ones-matmuls ----
            t_ps = tpool.tile([1, BC], F32)
            ones_f = wpool.tile([H, 1], F32)
            nc.vector.memset(ones_f[:], 1.0)
            for j in range(NCH):
                q_sb = wpool.tile([128, KFM, BC], BF16, name=f"q{j}", tag=f"q{j}")
                nc.vector.tensor_mul(q_sb[:], vt_tiles[j][:], z_tiles[j][:])
                for t in range(KFM):
                    nc.tensor.matmul(
                        t_ps[:], ones_sb[:], q_sb[:, t, :],
                        start=(j == 0 and t == 0), stop=False,
                    )
            nc.tensor.matmul(t_ps[:], woh_ap, h2_sb[:], start=False, stop=True)

            out_sb = wpool.tile([1, BC], F32)
            nc.scalar.activation(out_sb[:], t_ps[:], AF.Sigmoid, bias=bo_ap)
            nc.sync.dma_start(out_d[:, :], out_sb[:])

    nc.compile()
    _program_cache = nc
    return nc


def _prep_inputs(x, fm_w, w1, b1, w2, b2, wo, bo):
    x = np.asarray(x, dtype=np.float32)
    fm_w = np.asarray(fm_w, dtype=np.float32)
    w1 = np.asarray(w1, dtype=np.float32)
    w2 = np.asarray(w2, dtype=np.float32)
    wo = np.asarray(wo, dtype=np.float32).reshape(NP + H)
    b1 = np.asarray(b1, dtype=np.float32).reshape(H)
    b2 = np.asarray(b2, dtype=np.float32).reshape(H)
    bo = np.asarray(bo, dtype=np.float32).reshape(1)

    bf = ml_dtypes.bfloat16

    # Scatter pair weights into the strictly-upper triangle (static index
    # relayout, same (j1, j2>j1) row-major order as the reference), then
    # pack only the upper-triangular 128x128 blocks, j-major.
    wp = np.zeros((N, N), dtype=np.float32)
    wp[_IU1, _IU2] = wo[:NP]
    wp_bf = wp.astype(bf)
    wp_img = np.empty((128, WP_COLS), dtype=bf)
    for (k, j), off in UB_OFF.items():
        wp_img[:, off : off + 128] = wp_bf[
            128 * k : 128 * (k + 1), 128 * j : 128 * (j + 1)
        ]
    wp_img = np.ascontiguousarray(wp_img)

    w12_img = np.empty((128, NCH * H + H), dtype=bf)
    w12_img[:, : NCH * H] = _chunk_pack(w1.astype(bf), H)
    w12_img[:, NCH * H :] = w2.astype(bf)
    w12_img = np.ascontiguousarray(w12_img)

    f32_img = np.zeros((128, F32_COLS), dtype=np.float32)
    f32_img[:, FM_OFF : FM_OFF + NCH * KFM] = _chunk_pack(fm_w, KFM)
    f32_img[:, PK_OFF] = b1
    f32_img[:, PK_OFF + 1] = b2
    f32_img[:, PK_OFF + 2] = wo[NP:]
    f32_img[0, PK_OFF + 3] = bo[0]
    f32_img = np.ascontiguousarray(f32_img)

    xT = x.T.astype(bf)                                         # [512, 512]

    in_maps = []
    for c in range(N_CORES):
        xt_img = _chunk_pack(
            np.ascontiguousarray(xT[:, c * BC : (c + 1) * BC]), BC
        )                                                       # [128, 256]
        in_maps.append(
            {
                "wp": wp_img,
                "xtm": np.ascontiguousarray(xt_img),
                "w12": w12_img,
                "f32m": f32_img,
            }
        )
    return in_maps


def run(inputs, **spmd_kwargs):
    """Build, run on 8 cores, return (output [512,1] f32, BassKernelResults)."""
    nc = _build_program()
    in_maps = _prep_inputs(**inputs)
    res = run_bass_kernel_spmd(nc, in_maps, list(range(N_CORES)), **spmd_kwargs)
    out = np.concatenate(
        [res.results[c]["out"].reshape(BC) for c in range(N_CORES)]
    ).reshape(B, 1).astype(np.float32)
    return out, res


def kernel(**inputs) -> np.ndarray:
    out, _ = run(inputs)
    return out
